# revision 29
# baseline (speedup 1.0000x reference)
"""Multi-head causal attention (B=2, S=2048, D=1024, H=16, hd=64) on 8 trn2
NeuronCores.

Sharding: core c -> batch b=c//4, head-group g=c%4 (4 heads = 256 contiguous
model dims). Each core computes q/k/v projections for its head group from the
full (transposed) batch-b input, runs causal attention for its 4 heads, and
applies its slice of the output projection, producing a partial [2048, 1024]
output. The host sums the 4 partials per batch.

Speed structure (measured on HW, not guessed):
- All score matmuls contract over 128 partitions (q zero-padded per head via
  SBUF->SBUF DMA on idle DMA queues): 64-partition matmuls are hard-capped at
  the 1.2 GHz p-state; 128-partition ones ramp to 2.4 GHz.
- exp uses a global shift of 4 (cancels in softmax) so values fit fp8e4.
- Plain attention blocks for rows i>=512: AV matmul in fp8e4 DoubleRow mode
  (256-deep contraction = 2x bf16 FLOPs). exp produced by the Scalar engine
  (fp8 direct) or the DVE uint8 bit trick (bits = round(p/ln2+B), saturates
  at 0 -> +0.0, bitcast fp8e4).
- Diagonal bands: mask+exp folded into one DVE scalar_tensor_tensor
  (int16 bits = p*A + (B+mask), negative saturation -> -0.0), bitcast bf16.
- Rows i<512 keep true Scalar exp + bf16 AV (few-sample softmax rows cannot
  absorb fp8 noise).
- K and V projections for s>=512 and the s>=512 half of the output
  projection run as fp8 DoubleRow (hybrid keeps early rows accurate).
- Normalization avoids the PE: Scalar copies the denominator row, DVE
  reciprocal, gpsimd partition_broadcast, DVE multiply into z.
"""

import sys

for p in ("/opt/trn_rl_repo", "/root/.axon_site/_ro/trn_rl_repo"):
    if p not in sys.path:
        sys.path.insert(0, p)

import ml_dtypes
import numpy as np

B, S, DIM, H, HD = 2, 2048, 1024, 16, 64
NCORES = 8
HG = 4  # heads per core
OG = HG * HD  # 256 model dims per core
NB = S // 512  # 4 i-blocks of 512
NJ = S // 128  # 16 j-tiles of 128

LN2 = float(np.log(2.0))
SHIFT = 4.0  # global exp shift, cancels in softmax
A8 = 1.0 / LN2  # fp8 bit-trick slope (on raw psum p = 8*s)
B8 = 56.0 - 8.0 * SHIFT / LN2 - 0.344
A16 = 16.0 / LN2  # bf16 bit-trick slope (128*0.125/ln2)
B16 = 16256.0 - 128.0 * SHIFT / LN2 - 5.5

_CACHE = {}


def _build():
    import concourse.tile as tile
    from concourse import bacc, mybir

    f32 = mybir.dt.float32
    bf16 = mybir.dt.bfloat16
    f8 = mybir.dt.float8e4
    i16 = mybir.dt.int16
    u8 = mybir.dt.uint8
    Exp = mybir.ActivationFunctionType.Exp
    DR = mybir.MatmulPerfMode.DoubleRow
    MULT = mybir.AluOpType.mult
    ADD = mybir.AluOpType.add

    nc = bacc.Bacc("TRN2", target_bir_lowering=False, debug=False, num_devices=NCORES)

    xT = nc.dram_tensor("xT", [DIM, S], bf16, kind="ExternalInput")
    wqT = nc.dram_tensor("wqT", [DIM, OG], bf16, kind="ExternalInput")
    wkT = nc.dram_tensor("wkT", [DIM, OG], bf16, kind="ExternalInput")
    wkT8 = nc.dram_tensor("wkT8", [DIM, OG], f8, kind="ExternalInput")
    wvT = nc.dram_tensor("wvT", [DIM, OG], bf16, kind="ExternalInput")
    wvT8 = nc.dram_tensor("wvT8", [DIM, OG], f8, kind="ExternalInput")
    woT = nc.dram_tensor("woT", [OG, DIM], bf16, kind="ExternalInput")
    woT8 = nc.dram_tensor("woT8", [OG, DIM], f8, kind="ExternalInput")
    cmask8 = nc.dram_tensor("cmask8", [128, 128], f32, kind="ExternalInput")
    m16d = nc.dram_tensor("m16d", [128, 512], f32, kind="ExternalInput")
    y = nc.dram_tensor("y", [S, DIM], bf16, kind="ExternalOutput")

    wqT_r = wqT.ap().rearrange("(t p) o -> t p o", p=128)  # [8,128,256]
    wkT_r = wkT.ap().rearrange("(t p) o -> t p o", p=128)
    wvT_r = wvT.ap().rearrange("(t p) o -> t p o", p=128)
    woT_r = woT.ap().rearrange("(t p) e -> t p e", p=128)  # [2,128,1024]
    y_r = y.ap().rearrange("(t p) e -> t p e", p=128)  # [16,128,1024]
    xT_c = xT.ap().rearrange("(t p) s -> t p s", p=128)
    # fp8 DoubleRow layouts: k-subtile pairs along dim u
    wkT8_c = wkT8.ap().rearrange("(d u p) (m o) -> d m p u o", u=2, p=128, o=128)
    wvT8_c = wvT8.ap().rearrange("(d u p) o -> d p u o", u=2, p=128)
    woT8_c = woT8.ap().rearrange("(u p) (n e) -> n p u e", u=2, p=128, e=512)

    with tile.TileContext(nc) as tc:
        with (
            tc.tile_pool(name="persist", bufs=1) as pp,
            tc.tile_pool(name="work", bufs=4) as wp,
            tc.tile_pool(name="psum", bufs=4, space="PSUM") as ps,
        ):
            # ---- persistent SBUF tiles -------------------------------------
            xtb = [pp.tile([128, S], bf16, tag=f"xt{e}", name=f"xt{e}") for e in range(8)]
            xt2 = [[xtb[e][:, n * 512 : (n + 1) * 512] for n in range(4)] for e in range(8)]
            x8b = [pp.tile([128, 2, S], f8, tag=f"x8{d}", name=f"x8{d}") for d in range(4)]
            x8t = [[x8b[d][:, :, n * 512 : (n + 1) * 512] for n in range(4)] for d in range(4)]
            wqt = [pp.tile([128, OG], bf16, tag=f"wq{i}", name=f"wq{i}") for i in range(8)]
            wk8 = [
                [pp.tile([128, 2, 128], f8, tag=f"wk8{d}_{m}", name=f"wk8{d}_{m}") for m in range(2)]
                for d in range(4)
            ]
            wvt = [pp.tile([128, OG], bf16, tag=f"wv{i}", name=f"wv{i}") for i in range(8)]
            wv8 = [pp.tile([128, 2, OG], f8, tag=f"wv8{d}", name=f"wv8{d}") for d in range(4)]
            wot = [pp.tile([128, DIM], bf16, tag=f"wo{i}", name=f"wo{i}") for i in range(2)]
            wo8 = [pp.tile([128, 2, 512], f8, tag=f"wo8{i}", name=f"wo8{i}") for i in range(2)]
            cm = pp.tile([128, 128], f32, tag="cm")
            m16 = pp.tile([128, 512], f32, tag="m16")
            bias4 = pp.tile([128, 1], f32, tag="bias4")
            kT2 = [pp.tile([128, S], bf16, tag=f"kT{i}", name=f"kT{i}") for i in range(2)]
            qT2 = [pp.tile([128, S], bf16, tag=f"qT{i}", name=f"qT{i}") for i in range(2)]
            qTp = [pp.tile([128, S], bf16, tag=f"qTp{i}", name=f"qTp{i}") for i in range(4)]
            zT2 = [pp.tile([128, 512], bf16, tag=f"zT{i}", name=f"zT{i}") for i in range(2)]
            z8 = pp.tile([128, 12, 2, 128], f8, tag="z8")
            # bf16 V with ones column, per j-tile: [128, head, 65]
            vvb = [pp.tile([128, HG, HD + 1], bf16, tag=f"vvb{i}", name=f"vvb{i}") for i in range(NJ)]
            # fp8 V pairs for DoubleRow: [128, head, 2, 128]; col 64 = ones,
            # cols 65:128 = zeros (dual-fp8 ldweights wants full 128 cols)
            vv8 = [pp.tile([128, HG, 2, 128], f8, tag=f"vv8{i}", name=f"vv8{i}") for i in range(6)]

            qs = [nc.sync, nc.gpsimd, nc.scalar]
            qi = 0

            def q():
                nonlocal qi
                qi += 1
                return qs[qi % 3]

            wkt_full = [pp.tile([128, OG], bf16, tag=f"wkf{i}", name=f"wkf{i}") for i in range(8)]

            # ---- loads (ordered by first use; 3 queues, bulk x DMAs) -------
            for e in range(8):
                q().dma_start(out=xtb[e][:, 0:1024], in_=xT_c[e][:, 0:1024])
                q().dma_start(out=wkt_full[e], in_=wkT_r[e])
                q().dma_start(out=wqt[e], in_=wqT_r[e])
            for e in range(8):
                q().dma_start(out=wvt[e], in_=wvT_r[e])
                q().dma_start(out=xtb[e][:, 1024:2048], in_=xT_c[e][:, 1024:2048])
            for d in range(4):
                for m in range(2):
                    q().dma_start(out=wk8[d][m], in_=wkT8_c[d, m])
                q().dma_start(out=wv8[d], in_=wvT8_c[d])
            for i in range(2):
                q().dma_start(out=wot[i], in_=woT_r[i])
                q().dma_start(out=wo8[i], in_=woT8_c[i])
            nc.gpsimd.dma_start(out=cm, in_=cmask8.ap())
            nc.gpsimd.dma_start(out=m16, in_=m16d.ap())
            nc.vector.memset(bias4, -SHIFT)
            for i in range(6):
                nc.vector.memset(vv8[i], 0.0)
            # zero pads of qTp (the other head's partitions stay 0)
            for h in range(4):
                pad = slice(64, 128) if h % 2 == 0 else slice(0, 64)
                nc.vector.memset(qTp[h][pad, :], 0.0)
            # x8 derived on-chip (DVE idle during the load phase)
            for d in range(4):
                for u in range(2):
                    nc.vector.tensor_copy(x8b[d][:, u, 0:1024], xtb[2 * d + u][:, 0:1024])
            for d in range(4):
                for u in range(2):
                    nc.vector.tensor_copy(x8b[d][:, u, 1024:2048], xtb[2 * d + u][:, 1024:2048])

            def proj_qk(m, n, which):
                acc = ps.tile([128, 512], f32, tag="aux", bufs=2, name="acc")
                wt = wqt if which == "q" else wkt_full
                if which == "q" or n == 0:
                    for e in range(8):
                        nc.tensor.matmul(
                            acc,
                            wt[e][:, m * 128 : (m + 1) * 128],
                            xt2[e][n],
                            start=(e == 0),
                            stop=(e == 7),
                        )
                else:  # fp8 DoubleRow K
                    for d in range(4):
                        nc.tensor.matmul(
                            acc,
                            wk8[d][m],
                            x8t[d][n],
                            start=(d == 0),
                            stop=(d == 3),
                            perf_mode=DR,
                        )
                blk = slice(n * 512, (n + 1) * 512)
                dst = (kT2 if which == "k" else qT2)[m][:, blk]
                if (m + n) % 2 == 0:
                    nc.vector.tensor_copy(dst, acc)
                else:
                    nc.scalar.copy(dst, acc)
                if which == "q":
                    # padded per-head copies on idle DMA queues
                    for hh in (2 * m, 2 * m + 1):
                        rows = slice(0, 64) if hh % 2 == 0 else slice(64, 128)
                        nc.sync.dma_start(out=qTp[hh][rows, blk], in_=qT2[m][rows, blk])

            def proj_v(s):
                acc2 = ps.tile([128, 512], f32, tag="aux", bufs=2, name="acc2")
                av = acc2[:, 0:OG]
                if s < 4:
                    for e in range(8):
                        nc.tensor.matmul(
                            av,
                            xt2[e][s // 4][:, (s % 4) * 128 : (s % 4 + 1) * 128],
                            wvt[e],
                            start=(e == 0),
                            stop=(e == 7),
                        )
                else:
                    for d in range(4):
                        nc.tensor.matmul(
                            av,
                            x8t[d][s // 4][:, :, (s % 4) * 128 : (s % 4 + 1) * 128],
                            wv8[d],
                            start=(d == 0),
                            stop=(d == 3),
                            perf_mode=DR,
                        )
                vsrc = av.rearrange("p (h d) -> p h d", h=HG)
                nc.scalar.copy(vvb[s][:, :, 0:HD], vsrc)
                if s < 12:
                    nc.vector.tensor_copy(vv8[s // 2][:, :, s % 2, 0:HD], vsrc)
                nc.gpsimd.memset(vvb[s][:, :, HD : HD + 1], 1.0)
                if s < 12 and s % 2 == 1:
                    for u in range(2):
                        nc.gpsimd.memset(vv8[s // 2][:, :, u, HD : HD + 1], 1.0)

            # ---- attention -------------------------------------------------
            # exp-engine balance: fraction of plain pairs on Scalar
            sc_acc = [0.0]
            SC_SHARE = 0.55

            def mk_stream(ib, h):
                m = h // 2
                pslc = slice(64 * (h % 2), 64 * (h % 2) + 64)
                iblk = slice(ib * 512, (ib + 1) * 512)
                psz = ps.tile([128, 512], f32, tag="z", bufs=2, name="psz")
                units = []

                for pr in range(2 * ib):  # plain j-tile pairs, fp8 DoubleRow
                    def mk(pr=pr):
                        ex = wp.tile([128, 2, 512], f8, tag="ex8", bufs=12, name="ex8")
                        sc_acc[0] += SC_SHARE
                        on_scalar = sc_acc[0] >= 1.0
                        if on_scalar:
                            sc_acc[0] -= 1.0

                        def se():
                            for u in range(2):
                                psp = ps.tile([128, 512], f32, tag="sc", bufs=4, name="psp")
                                nc.tensor.matmul(
                                    psp,
                                    kT2[m][:, (2 * pr + u) * 128 : (2 * pr + u + 1) * 128],
                                    qTp[h][:, iblk],
                                    start=True,
                                    stop=True,
                                )
                                if on_scalar:
                                    nc.scalar.activation(
                                        ex[:, u, :], psp, Exp, scale=0.125, bias=bias4
                                    )
                                else:
                                    nc.vector.tensor_scalar(
                                        ex.bitcast(u8)[:, u, :], psp, A8, B8, MULT, ADD
                                    )

                        def av():
                            nc.tensor.matmul(
                                psz,
                                vv8[pr][:, h, :, :],
                                ex,
                                start=(pr == 0),
                                stop=False,
                                perf_mode=DR,
                            )

                        return se, av

                    units.append(mk())

                for t in range(4):  # diagonal band tiles, bf16
                    def mk(t=t):
                        jb = 4 * ib + t
                        off = 128 * t
                        w = 512 - off
                        exb = wp.tile([128, 512], i16, tag="exb", bufs=12, name="exb")

                        def se():
                            ps2 = ps.tile([128, 512], f32, tag="sc", bufs=4, name="ps2")
                            nc.tensor.matmul(
                                ps2[:, off:512],
                                kT2[m][:, jb * 128 : (jb + 1) * 128],
                                qTp[h][:, ib * 512 + off : (ib + 1) * 512],
                                start=True,
                                stop=True,
                            )
                            if ib == 0:
                                # precision-critical rows: true exp (bf16 out)
                                nc.vector.tensor_add(
                                    ps2[:, off : off + 128],
                                    ps2[:, off : off + 128],
                                    cm,
                                )
                                nc.scalar.activation(
                                    exb.bitcast(bf16)[:, off:512],
                                    ps2[:, off:512],
                                    Exp,
                                    scale=0.125,
                                    bias=bias4,
                                )
                            else:
                                # mask + exp folded into one DVE op
                                nc.vector.scalar_tensor_tensor(
                                    out=exb[:, off:512],
                                    in0=ps2[:, off:512],
                                    scalar=A16,
                                    in1=m16[:, 0:w],
                                    op0=MULT,
                                    op1=ADD,
                                )

                        def av():
                            nc.tensor.matmul(
                                psz[0:65, off:512],
                                vvb[jb][:, h, :],
                                exb.bitcast(bf16)[:, off:512],
                                start=(ib == 0 and t == 0),
                                stop=(t == 3),
                            )

                        return se, av

                    units.append(mk())

                def norm():
                    dn = wp.tile([1, 512], f32, tag="dn", bufs=4, name="dn")
                    nc.scalar.copy(dn, psz[64:65, :])
                    rc1 = wp.tile([1, 512], f32, tag="rc1", bufs=4, name="rc1")
                    nc.vector.reciprocal_approx_fast(rc1, dn)
                    rcb = wp.tile([64, 512], f32, tag="rcb", bufs=4, name="rcb")
                    nc.gpsimd.partition_broadcast(rcb, rc1)
                    if ib == 0:
                        nc.vector.tensor_mul(zT2[m][pslc, :], psz[0:64, :], rcb)
                    else:
                        nc.vector.tensor_mul(
                            z8[pslc, 4 * ib - 4 : 4 * ib, m, :],
                            psz[0:64, :].rearrange("p (s c) -> p s c", c=128),
                            rcb.rearrange("p (s c) -> p s c", c=128),
                        )

                return units, norm

            def mk_outproj_s(s):
                def go():
                    ysb = wp.tile([128, DIM], bf16, tag="ysb", bufs=3, name="ysb")
                    for n2 in range(2):
                        psy = ps.tile([128, 512], f32, tag="aux", bufs=2, name="psy")
                        if s < 4:
                            for kk in range(2):
                                nc.tensor.matmul(
                                    psy,
                                    zT2[kk][:, s * 128 : (s + 1) * 128],
                                    wot[kk][:, n2 * 512 : (n2 + 1) * 512],
                                    start=(kk == 0),
                                    stop=(kk == 1),
                                )
                        else:
                            nc.tensor.matmul(
                                psy,
                                z8[:, s - 4, :, :],
                                wo8[n2],
                                start=True,
                                stop=True,
                                perf_mode=DR,
                            )
                        half = ysb[:, n2 * 512 : (n2 + 1) * 512]
                        if (s + n2) % 2 == 0:
                            nc.scalar.copy(half, psy)
                        else:
                            nc.vector.tensor_copy(half, psy)
                    nc.sync.dma_start(out=y_r[s], in_=ysb)

                return go

            # cross-stream software pipeline: a global action FIFO keeps the
            # PE's AV/norm/outproj work LAG score-units behind the score/exp
            # emissions so exp results are ready when the PE needs them.
            LAG = 5
            fifo = []

            def push(action):
                fifo.append(action)
                while len(fifo) > LAG:
                    fifo.pop(0)()

            # minimal prologue: only what stream (0,h0) needs
            proj_qk(0, 0, "k")
            proj_qk(0, 0, "q")
            for s in range(4):
                proj_v(s)
            # remaining proj work interleaved into early attention streams
            proj_actions = [
                [lambda: proj_qk(1, 0, "k"), lambda: proj_qk(1, 0, "q")],
                [lambda: proj_qk(0, 1, "k"), lambda: proj_qk(0, 1, "q")],
                [lambda: proj_qk(1, 1, "k"), lambda: proj_qk(1, 1, "q"),
                 lambda: proj_v(4), lambda: proj_v(5)],
                [lambda: proj_qk(0, 2, "k"), lambda: proj_qk(0, 2, "q"),
                 lambda: proj_v(6), lambda: proj_v(7)],
                [lambda: proj_qk(1, 2, "k"), lambda: proj_qk(1, 2, "q"),
                 lambda: proj_v(8), lambda: proj_v(9)],
                [lambda: proj_qk(0, 3, "k"), lambda: proj_qk(0, 3, "q"),
                 lambda: proj_v(10), lambda: proj_v(11)],
                [lambda: proj_qk(1, 3, "k"), lambda: proj_qk(1, 3, "q"),
                 lambda: proj_v(12), lambda: proj_v(13)],
                [lambda: proj_v(14), lambda: proj_v(15)],
            ]
            ready_outproj = []
            sidx = 0
            for ib in range(NB):
                for h in range(HG):
                    units, norm = mk_stream(ib, h)
                    acts = list(proj_actions[sidx]) if sidx < len(proj_actions) else []
                    # feed pending outproj units into the tail streams too
                    while ready_outproj and len(acts) < max(2, len(units) // 3):
                        acts.append(ready_outproj.pop(0))
                    na = len(units)
                    step = max(1, na // (len(acts) + 1)) if acts else na + 1
                    for ui, (se, av) in enumerate(units):
                        se()
                        push(av)
                        if acts and ui % step == step - 1:
                            push(acts.pop(0))
                    for a in acts:
                        push(a)
                    push(norm)
                    sidx += 1
                    if h == 3 and ib >= 1:
                        for s in range(4 * (ib - 1), 4 * ib):
                            ready_outproj.append(mk_outproj_s(s))
            while fifo:
                fifo.pop(0)()
            for a in ready_outproj:
                a()
            for s in range(12, 16):
                mk_outproj_s(s)()

    nc.compile()
    return nc


def _get_nc():
    if "nc" not in _CACHE:
        _CACHE["nc"] = _build()
    return _CACHE["nc"]


def _in_maps(x, mask, wq, wk, wv, wo):
    bf = ml_dtypes.bfloat16
    e4 = ml_dtypes.float8_e4m3
    cm8 = np.ascontiguousarray(8.0 * np.asarray(mask)[0, 0, :128, :128].T, np.float32)
    m16 = np.full((128, 512), B16, np.float32)
    tri = np.asarray(mask)[0, 0, :128, :128].T  # [j, i], -1e9 where j > i
    m16[:, :128] += tri.astype(np.float32)
    maps = []
    for c in range(NCORES):
        b, g = divmod(c, HG)
        sl = slice(OG * g, OG * (g + 1))
        xTb = np.ascontiguousarray(np.asarray(x)[b].T)
        wkTg = np.ascontiguousarray(np.asarray(wk)[sl, :].T)
        wvTg = np.ascontiguousarray(np.asarray(wv)[sl, :].T)
        woTg = np.ascontiguousarray(np.asarray(wo)[:, sl].T)
        maps.append(
            {
                "xT": xTb.astype(bf),
                "wqT": np.ascontiguousarray(np.asarray(wq)[sl, :].T).astype(bf),
                "wkT": wkTg.astype(bf),
                "wkT8": wkTg.astype(e4),
                "wvT": wvTg.astype(bf),
                "wvT8": wvTg.astype(e4),
                "woT": woTg.astype(bf),
                "woT8": woTg.astype(e4),
                "cmask8": cm8,
                "m16d": m16,
            }
        )
    return maps


def _combine(results):
    y = np.zeros((B, S, DIM), np.float32)
    for c in range(NCORES):
        y[c // HG] += results[c]["y"].astype(np.float32)
    return y


def kernel(x, mask, wq, wk, wv, wo, **run_kwargs):
    from concourse.bass_utils import run_bass_kernel_spmd

    nc = _get_nc()
    res = run_bass_kernel_spmd(
        nc, _in_maps(x, mask, wq, wk, wv, wo), core_ids=list(range(NCORES)),
        **run_kwargs,
    )
    out = _combine(res.results)
    if run_kwargs:
        _CACHE["last_result"] = res
    return out


# revision 30
# speedup vs baseline: 1.0283x; 1.0283x over previous
"""Multi-head causal attention (B=2, S=2048, D=1024, H=16, hd=64) on 8 trn2
NeuronCores.

Sharding: core c -> batch b=c//4, head-group g=c%4 (4 heads = 256 contiguous
model dims). Each core computes q/k/v projections for its head group from the
full (transposed) batch-b input, runs causal attention for its 4 heads, and
applies its slice of the output projection, producing a partial [2048, 1024]
output. The host sums the 4 partials per batch.

Speed structure (measured on HW, not guessed):
- All score matmuls contract over 128 partitions (q zero-padded per head via
  SBUF->SBUF DMA on idle DMA queues): 64-partition matmuls are hard-capped at
  the 1.2 GHz p-state; 128-partition ones ramp to 2.4 GHz.
- exp uses a global shift of 4 (cancels in softmax) so values fit fp8e4.
- Plain attention blocks for rows i>=512: AV matmul in fp8e4 DoubleRow mode
  (256-deep contraction = 2x bf16 FLOPs). exp produced by the Scalar engine
  (fp8 direct) or the DVE uint8 bit trick (bits = round(p/ln2+B), saturates
  at 0 -> +0.0, bitcast fp8e4).
- Diagonal bands: mask+exp folded into one DVE scalar_tensor_tensor
  (int16 bits = p*A + (B+mask), negative saturation -> -0.0), bitcast bf16.
- Rows i<512 keep true Scalar exp + bf16 AV (few-sample softmax rows cannot
  absorb fp8 noise).
- K and V projections for s>=512 and the s>=512 half of the output
  projection run as fp8 DoubleRow (hybrid keeps early rows accurate).
- Normalization avoids the PE: Scalar copies the denominator row, DVE
  reciprocal, gpsimd partition_broadcast, DVE multiply into z.
"""

import sys

for p in ("/opt/trn_rl_repo", "/root/.axon_site/_ro/trn_rl_repo"):
    if p not in sys.path:
        sys.path.insert(0, p)

import ml_dtypes
import numpy as np

B, S, DIM, H, HD = 2, 2048, 1024, 16, 64
NCORES = 8
HG = 4  # heads per core
OG = HG * HD  # 256 model dims per core
NB = S // 512  # 4 i-blocks of 512
NJ = S // 128  # 16 j-tiles of 128

LN2 = float(np.log(2.0))
SHIFT = 4.0  # global exp shift, cancels in softmax
A8 = 1.0 / LN2  # fp8 bit-trick slope (on raw psum p = 8*s)
B8 = 56.0 - 8.0 * SHIFT / LN2 - 0.344
A16 = 16.0 / LN2  # bf16 bit-trick slope (128*0.125/ln2)
B16 = 16256.0 - 128.0 * SHIFT / LN2 - 5.5

_CACHE = {}


def _build():
    import concourse.tile as tile
    from concourse import bacc, mybir

    f32 = mybir.dt.float32
    bf16 = mybir.dt.bfloat16
    f8 = mybir.dt.float8e4
    i16 = mybir.dt.int16
    u8 = mybir.dt.uint8
    Exp = mybir.ActivationFunctionType.Exp
    DR = mybir.MatmulPerfMode.DoubleRow
    MULT = mybir.AluOpType.mult
    ADD = mybir.AluOpType.add

    nc = bacc.Bacc("TRN2", target_bir_lowering=False, debug=False, num_devices=NCORES)

    xT = nc.dram_tensor("xT", [DIM, S], bf16, kind="ExternalInput")
    wqT = nc.dram_tensor("wqT", [DIM, OG], bf16, kind="ExternalInput")
    wkT = nc.dram_tensor("wkT", [DIM, OG], bf16, kind="ExternalInput")
    wkT8 = nc.dram_tensor("wkT8", [DIM, OG], f8, kind="ExternalInput")
    wvT = nc.dram_tensor("wvT", [DIM, OG], bf16, kind="ExternalInput")
    wvT8 = nc.dram_tensor("wvT8", [DIM, OG], f8, kind="ExternalInput")
    woT = nc.dram_tensor("woT", [OG, DIM], bf16, kind="ExternalInput")
    woT8 = nc.dram_tensor("woT8", [OG, DIM], f8, kind="ExternalInput")
    cmask8 = nc.dram_tensor("cmask8", [128, 128], f32, kind="ExternalInput")
    m16d = nc.dram_tensor("m16d", [128, 512], f32, kind="ExternalInput")
    y = nc.dram_tensor("y", [S, DIM], bf16, kind="ExternalOutput")

    wqT_r = wqT.ap().rearrange("(t p) o -> t p o", p=128)  # [8,128,256]
    wkT_r = wkT.ap().rearrange("(t p) o -> t p o", p=128)
    wvT_r = wvT.ap().rearrange("(t p) o -> t p o", p=128)
    woT_r = woT.ap().rearrange("(t p) e -> t p e", p=128)  # [2,128,1024]
    y_r = y.ap().rearrange("(t p) e -> t p e", p=128)  # [16,128,1024]
    xT_c = xT.ap().rearrange("(t p) s -> t p s", p=128)
    # fp8 DoubleRow layouts: k-subtile pairs along dim u
    wkT8_c = wkT8.ap().rearrange("(d u p) (m o) -> d m p u o", u=2, p=128, o=128)
    wvT8_c = wvT8.ap().rearrange("(d u p) o -> d p u o", u=2, p=128)
    woT8_c = woT8.ap().rearrange("(u p) (n e) -> n p u e", u=2, p=128, e=512)

    with tile.TileContext(nc) as tc:
        with (
            tc.tile_pool(name="persist", bufs=1) as pp,
            tc.tile_pool(name="work", bufs=4) as wp,
            tc.tile_pool(name="psum", bufs=4, space="PSUM") as ps,
        ):
            # ---- persistent SBUF tiles -------------------------------------
            xtb = [pp.tile([128, S], bf16, tag=f"xt{e}", name=f"xt{e}") for e in range(8)]
            xt2 = [[xtb[e][:, n * 512 : (n + 1) * 512] for n in range(4)] for e in range(8)]
            x8b = [pp.tile([128, 2, S], f8, tag=f"x8{d}", name=f"x8{d}") for d in range(4)]
            x8t = [[x8b[d][:, :, n * 512 : (n + 1) * 512] for n in range(4)] for d in range(4)]
            wqt = [pp.tile([128, OG], bf16, tag=f"wq{i}", name=f"wq{i}") for i in range(8)]
            wk8 = [
                [pp.tile([128, 2, 128], f8, tag=f"wk8{d}_{m}", name=f"wk8{d}_{m}") for m in range(2)]
                for d in range(4)
            ]
            wvt = [pp.tile([128, OG], bf16, tag=f"wv{i}", name=f"wv{i}") for i in range(8)]
            wv8 = [pp.tile([128, 2, OG], f8, tag=f"wv8{d}", name=f"wv8{d}") for d in range(4)]
            wot = [pp.tile([128, DIM], bf16, tag=f"wo{i}", name=f"wo{i}") for i in range(2)]
            wo8 = [pp.tile([128, 2, 512], f8, tag=f"wo8{i}", name=f"wo8{i}") for i in range(2)]
            cm = pp.tile([128, 128], f32, tag="cm")
            m16 = pp.tile([128, 512], f32, tag="m16")
            bias4 = pp.tile([128, 1], f32, tag="bias4")
            kT2 = [pp.tile([128, S], bf16, tag=f"kT{i}", name=f"kT{i}") for i in range(2)]
            qT2 = [pp.tile([128, S], bf16, tag=f"qT{i}", name=f"qT{i}") for i in range(2)]
            qTp = [pp.tile([128, S], bf16, tag=f"qTp{i}", name=f"qTp{i}") for i in range(4)]
            zT2 = [pp.tile([128, 512], bf16, tag=f"zT{i}", name=f"zT{i}") for i in range(2)]
            z8 = pp.tile([128, 12, 2, 128], f8, tag="z8")
            # bf16 V with ones column, per j-tile: [128, head, 65]
            vvb = [pp.tile([128, HG, HD + 1], bf16, tag=f"vvb{i}", name=f"vvb{i}") for i in range(NJ)]
            # fp8 V pairs for DoubleRow: [128, head, 2, 128]; col 64 = ones,
            # cols 65:128 = zeros (dual-fp8 ldweights wants full 128 cols)
            vv8 = [pp.tile([128, HG, 2, 128], f8, tag=f"vv8{i}", name=f"vv8{i}") for i in range(6)]

            qs = [nc.sync, nc.gpsimd, nc.scalar]
            qi = 0

            def q():
                nonlocal qi
                qi += 1
                return qs[qi % 3]

            wkt_full = [pp.tile([128, OG], bf16, tag=f"wkf{i}", name=f"wkf{i}") for i in range(8)]

            # ---- loads (ordered by first use; 3 queues, bulk x DMAs) -------
            for e in range(8):
                q().dma_start(out=xtb[e][:, 0:1024], in_=xT_c[e][:, 0:1024])
                q().dma_start(out=wkt_full[e], in_=wkT_r[e])
                q().dma_start(out=wqt[e], in_=wqT_r[e])
            for e in range(8):
                q().dma_start(out=wvt[e], in_=wvT_r[e])
                q().dma_start(out=xtb[e][:, 1024:2048], in_=xT_c[e][:, 1024:2048])
            for d in range(4):
                for m in range(2):
                    q().dma_start(out=wk8[d][m], in_=wkT8_c[d, m])
                q().dma_start(out=wv8[d], in_=wvT8_c[d])
            for i in range(2):
                q().dma_start(out=wot[i], in_=woT_r[i])
                q().dma_start(out=wo8[i], in_=woT8_c[i])
            nc.gpsimd.dma_start(out=cm, in_=cmask8.ap())
            nc.gpsimd.dma_start(out=m16, in_=m16d.ap())
            nc.vector.memset(bias4, -SHIFT)
            for i in range(6):
                nc.vector.memset(vv8[i], 0.0)
            # zero pads of qTp (the other head's partitions stay 0)
            for h in range(4):
                pad = slice(64, 128) if h % 2 == 0 else slice(0, 64)
                nc.vector.memset(qTp[h][pad, :], 0.0)
            # x8 derived on-chip (DVE idle during the load phase)
            for d in range(4):
                for u in range(2):
                    nc.vector.tensor_copy(x8b[d][:, u, 0:1024], xtb[2 * d + u][:, 0:1024])
            for d in range(4):
                for u in range(2):
                    nc.vector.tensor_copy(x8b[d][:, u, 1024:2048], xtb[2 * d + u][:, 1024:2048])

            def proj_qk(m, n, which):
                acc = ps.tile([128, 512], f32, tag="aux", bufs=2, name="acc")
                wt = wqt if which == "q" else wkt_full
                if which == "q" or n == 0:
                    for e in range(8):
                        nc.tensor.matmul(
                            acc,
                            wt[e][:, m * 128 : (m + 1) * 128],
                            xt2[e][n],
                            start=(e == 0),
                            stop=(e == 7),
                        )
                else:  # fp8 DoubleRow K
                    for d in range(4):
                        nc.tensor.matmul(
                            acc,
                            wk8[d][m],
                            x8t[d][n],
                            start=(d == 0),
                            stop=(d == 3),
                            perf_mode=DR,
                        )
                blk = slice(n * 512, (n + 1) * 512)
                dst = (kT2 if which == "k" else qT2)[m][:, blk]
                if (m + n) % 2 == 0:
                    nc.vector.tensor_copy(dst, acc)
                else:
                    nc.scalar.copy(dst, acc)
                if which == "q":
                    # padded per-head copies on idle DMA queues
                    for hh in (2 * m, 2 * m + 1):
                        rows = slice(0, 64) if hh % 2 == 0 else slice(64, 128)
                        nc.sync.dma_start(out=qTp[hh][rows, blk], in_=qT2[m][rows, blk])

            def proj_v(s):
                acc2 = ps.tile([128, 512], f32, tag="aux", bufs=2, name="acc2")
                av = acc2[:, 0:OG]
                if s < 4:
                    for e in range(8):
                        nc.tensor.matmul(
                            av,
                            xt2[e][s // 4][:, (s % 4) * 128 : (s % 4 + 1) * 128],
                            wvt[e],
                            start=(e == 0),
                            stop=(e == 7),
                        )
                else:
                    for d in range(4):
                        nc.tensor.matmul(
                            av,
                            x8t[d][s // 4][:, :, (s % 4) * 128 : (s % 4 + 1) * 128],
                            wv8[d],
                            start=(d == 0),
                            stop=(d == 3),
                            perf_mode=DR,
                        )
                vsrc = av.rearrange("p (h d) -> p h d", h=HG)
                nc.scalar.copy(vvb[s][:, :, 0:HD], vsrc)
                if s < 12:
                    nc.vector.tensor_copy(vv8[s // 2][:, :, s % 2, 0:HD], vsrc)
                nc.gpsimd.memset(vvb[s][:, :, HD : HD + 1], 1.0)
                if s < 12 and s % 2 == 1:
                    for u in range(2):
                        nc.gpsimd.memset(vv8[s // 2][:, :, u, HD : HD + 1], 1.0)

            # ---- attention -------------------------------------------------
            # exp-engine balance: fraction of plain pairs on Scalar
            sc_acc = [0.0]
            SC_SHARE = 0.80

            def mk_stream(ib, h):
                m = h // 2
                pslc = slice(64 * (h % 2), 64 * (h % 2) + 64)
                iblk = slice(ib * 512, (ib + 1) * 512)
                psz = ps.tile([128, 512], f32, tag="z", bufs=2, name="psz")
                units = []

                for pr in range(2 * ib):  # plain j-tile pairs, fp8 DoubleRow
                    def mk(pr=pr):
                        ex = wp.tile([128, 2, 512], f8, tag="ex8", bufs=12, name="ex8")
                        sc_acc[0] += SC_SHARE
                        on_scalar = sc_acc[0] >= 1.0
                        if on_scalar:
                            sc_acc[0] -= 1.0

                        def se():
                            for u in range(2):
                                psp = ps.tile([128, 512], f32, tag="sc", bufs=4, name="psp")
                                nc.tensor.matmul(
                                    psp,
                                    kT2[m][:, (2 * pr + u) * 128 : (2 * pr + u + 1) * 128],
                                    qTp[h][:, iblk],
                                    start=True,
                                    stop=True,
                                )
                                if on_scalar:
                                    nc.scalar.activation(
                                        ex[:, u, :], psp, Exp, scale=0.125, bias=bias4
                                    )
                                else:
                                    nc.vector.tensor_scalar(
                                        ex.bitcast(u8)[:, u, :], psp, A8, B8, MULT, ADD
                                    )

                        def av():
                            nc.tensor.matmul(
                                psz,
                                vv8[pr][:, h, :, :],
                                ex,
                                start=(pr == 0),
                                stop=False,
                                perf_mode=DR,
                            )

                        return se, av

                    units.append(mk())

                for t in range(4):  # diagonal band tiles, bf16
                    def mk(t=t):
                        jb = 4 * ib + t
                        off = 128 * t
                        w = 512 - off
                        exb = wp.tile([128, 512], i16, tag="exb", bufs=12, name="exb")

                        def se():
                            ps2 = ps.tile([128, 512], f32, tag="sc", bufs=4, name="ps2")
                            nc.tensor.matmul(
                                ps2[:, off:512],
                                kT2[m][:, jb * 128 : (jb + 1) * 128],
                                qTp[h][:, ib * 512 + off : (ib + 1) * 512],
                                start=True,
                                stop=True,
                            )
                            if ib == 0:
                                # precision-critical rows: true exp (bf16 out)
                                nc.vector.tensor_add(
                                    ps2[:, off : off + 128],
                                    ps2[:, off : off + 128],
                                    cm,
                                )
                                nc.scalar.activation(
                                    exb.bitcast(bf16)[:, off:512],
                                    ps2[:, off:512],
                                    Exp,
                                    scale=0.125,
                                    bias=bias4,
                                )
                            else:
                                # mask + exp folded into one DVE op
                                nc.vector.scalar_tensor_tensor(
                                    out=exb[:, off:512],
                                    in0=ps2[:, off:512],
                                    scalar=A16,
                                    in1=m16[:, 0:w],
                                    op0=MULT,
                                    op1=ADD,
                                )

                        def av():
                            nc.tensor.matmul(
                                psz[0:65, off:512],
                                vvb[jb][:, h, :],
                                exb.bitcast(bf16)[:, off:512],
                                start=(ib == 0 and t == 0),
                                stop=(t == 3),
                            )

                        return se, av

                    units.append(mk())

                def norm():
                    dn = wp.tile([1, 512], f32, tag="dn", bufs=4, name="dn")
                    nc.scalar.copy(dn, psz[64:65, :])
                    rc1 = wp.tile([1, 512], f32, tag="rc1", bufs=4, name="rc1")
                    nc.vector.reciprocal_approx_fast(rc1, dn)
                    rcb = wp.tile([64, 512], f32, tag="rcb", bufs=4, name="rcb")
                    nc.gpsimd.partition_broadcast(rcb, rc1)
                    if ib == 0:
                        nc.vector.tensor_mul(zT2[m][pslc, :], psz[0:64, :], rcb)
                    else:
                        nc.vector.tensor_mul(
                            z8[pslc, 4 * ib - 4 : 4 * ib, m, :],
                            psz[0:64, :].rearrange("p (s c) -> p s c", c=128),
                            rcb.rearrange("p (s c) -> p s c", c=128),
                        )

                return units, norm

            def mk_outproj_s(s):
                def go():
                    ysb = wp.tile([128, DIM], bf16, tag="ysb", bufs=3, name="ysb")
                    for n2 in range(2):
                        psy = ps.tile([128, 512], f32, tag="aux", bufs=2, name="psy")
                        if s < 4:
                            for kk in range(2):
                                nc.tensor.matmul(
                                    psy,
                                    zT2[kk][:, s * 128 : (s + 1) * 128],
                                    wot[kk][:, n2 * 512 : (n2 + 1) * 512],
                                    start=(kk == 0),
                                    stop=(kk == 1),
                                )
                        else:
                            nc.tensor.matmul(
                                psy,
                                z8[:, s - 4, :, :],
                                wo8[n2],
                                start=True,
                                stop=True,
                                perf_mode=DR,
                            )
                        half = ysb[:, n2 * 512 : (n2 + 1) * 512]
                        if (s + n2) % 2 == 0:
                            nc.scalar.copy(half, psy)
                        else:
                            nc.vector.tensor_copy(half, psy)
                    nc.sync.dma_start(out=y_r[s], in_=ysb)

                return go

            # cross-stream software pipeline: a global action FIFO keeps the
            # PE's AV/norm/outproj work LAG score-units behind the score/exp
            # emissions so exp results are ready when the PE needs them.
            LAG = 5
            fifo = []

            def push(action):
                fifo.append(action)
                while len(fifo) > LAG:
                    fifo.pop(0)()

            # minimal prologue: only what stream (0,h0) needs
            proj_qk(0, 0, "k")
            proj_qk(0, 0, "q")
            for s in range(4):
                proj_v(s)
            # remaining proj work interleaved into early attention streams
            proj_actions = [
                [lambda: proj_qk(1, 0, "k"), lambda: proj_qk(1, 0, "q")],
                [lambda: proj_qk(0, 1, "k"), lambda: proj_qk(0, 1, "q")],
                [lambda: proj_qk(1, 1, "k"), lambda: proj_qk(1, 1, "q"),
                 lambda: proj_v(4), lambda: proj_v(5)],
                [lambda: proj_qk(0, 2, "k"), lambda: proj_qk(0, 2, "q"),
                 lambda: proj_v(6), lambda: proj_v(7)],
                [lambda: proj_qk(1, 2, "k"), lambda: proj_qk(1, 2, "q"),
                 lambda: proj_v(8), lambda: proj_v(9)],
                [lambda: proj_qk(0, 3, "k"), lambda: proj_qk(0, 3, "q"),
                 lambda: proj_v(10), lambda: proj_v(11)],
                [lambda: proj_qk(1, 3, "k"), lambda: proj_qk(1, 3, "q"),
                 lambda: proj_v(12), lambda: proj_v(13)],
                [lambda: proj_v(14), lambda: proj_v(15)],
            ]
            ready_outproj = []
            sidx = 0
            for ib in range(NB):
                for h in range(HG):
                    units, norm = mk_stream(ib, h)
                    acts = list(proj_actions[sidx]) if sidx < len(proj_actions) else []
                    # feed pending outproj units into the tail streams too
                    while ready_outproj and len(acts) < max(2, len(units) // 3):
                        acts.append(ready_outproj.pop(0))
                    na = len(units)
                    step = max(1, na // (len(acts) + 1)) if acts else na + 1
                    for ui, (se, av) in enumerate(units):
                        se()
                        push(av)
                        if acts and ui % step == step - 1:
                            push(acts.pop(0))
                    for a in acts:
                        push(a)
                    push(norm)
                    sidx += 1
                    if h == 3 and ib >= 1:
                        for s in range(4 * (ib - 1), 4 * ib):
                            ready_outproj.append(mk_outproj_s(s))
            while fifo:
                fifo.pop(0)()
            for a in ready_outproj:
                a()
            for s in range(12, 16):
                mk_outproj_s(s)()

    nc.compile()
    return nc


def _get_nc():
    if "nc" not in _CACHE:
        _CACHE["nc"] = _build()
    return _CACHE["nc"]


def _in_maps(x, mask, wq, wk, wv, wo):
    bf = ml_dtypes.bfloat16
    e4 = ml_dtypes.float8_e4m3
    cm8 = np.ascontiguousarray(8.0 * np.asarray(mask)[0, 0, :128, :128].T, np.float32)
    m16 = np.full((128, 512), B16, np.float32)
    tri = np.asarray(mask)[0, 0, :128, :128].T  # [j, i], -1e9 where j > i
    m16[:, :128] += tri.astype(np.float32)
    maps = []
    for c in range(NCORES):
        b, g = divmod(c, HG)
        sl = slice(OG * g, OG * (g + 1))
        xTb = np.ascontiguousarray(np.asarray(x)[b].T)
        wkTg = np.ascontiguousarray(np.asarray(wk)[sl, :].T)
        wvTg = np.ascontiguousarray(np.asarray(wv)[sl, :].T)
        woTg = np.ascontiguousarray(np.asarray(wo)[:, sl].T)
        maps.append(
            {
                "xT": xTb.astype(bf),
                "wqT": np.ascontiguousarray(np.asarray(wq)[sl, :].T).astype(bf),
                "wkT": wkTg.astype(bf),
                "wkT8": wkTg.astype(e4),
                "wvT": wvTg.astype(bf),
                "wvT8": wvTg.astype(e4),
                "woT": woTg.astype(bf),
                "woT8": woTg.astype(e4),
                "cmask8": cm8,
                "m16d": m16,
            }
        )
    return maps


def _combine(results):
    y = np.zeros((B, S, DIM), np.float32)
    for c in range(NCORES):
        y[c // HG] += results[c]["y"].astype(np.float32)
    return y


def kernel(x, mask, wq, wk, wv, wo, **run_kwargs):
    from concourse.bass_utils import run_bass_kernel_spmd

    nc = _get_nc()
    res = run_bass_kernel_spmd(
        nc, _in_maps(x, mask, wq, wk, wv, wo), core_ids=list(range(NCORES)),
        **run_kwargs,
    )
    out = _combine(res.results)
    if run_kwargs:
        _CACHE["last_result"] = res
    return out


# revision 31
# speedup vs baseline: 1.0309x; 1.0026x over previous
"""Multi-head causal attention (B=2, S=2048, D=1024, H=16, hd=64) on 8 trn2
NeuronCores.

Sharding: core c -> batch b=c//4, head-group g=c%4 (4 heads = 256 contiguous
model dims). Each core computes q/k/v projections for its head group from the
full (transposed) batch-b input, runs causal attention for its 4 heads, and
applies its slice of the output projection, producing a partial [2048, 1024]
output. The host sums the 4 partials per batch.

Speed structure (measured on HW, not guessed):
- All score matmuls contract over 128 partitions (q zero-padded per head via
  SBUF->SBUF DMA on idle DMA queues): 64-partition matmuls are hard-capped at
  the 1.2 GHz p-state; 128-partition ones ramp to 2.4 GHz.
- exp uses a global shift of 4 (cancels in softmax) so values fit fp8e4.
- Plain attention blocks for rows i>=512: AV matmul in fp8e4 DoubleRow mode
  (256-deep contraction = 2x bf16 FLOPs). exp produced by the Scalar engine
  (fp8 direct) or the DVE uint8 bit trick (bits = round(p/ln2+B), saturates
  at 0 -> +0.0, bitcast fp8e4).
- Diagonal bands: mask+exp folded into one DVE scalar_tensor_tensor
  (int16 bits = p*A + (B+mask), negative saturation -> -0.0), bitcast bf16.
- Rows i<512 keep true Scalar exp + bf16 AV (few-sample softmax rows cannot
  absorb fp8 noise).
- K and V projections for s>=512 and the s>=512 half of the output
  projection run as fp8 DoubleRow (hybrid keeps early rows accurate).
- Normalization avoids the PE: Scalar copies the denominator row, DVE
  reciprocal, gpsimd partition_broadcast, DVE multiply into z.
"""

import sys

for p in ("/opt/trn_rl_repo", "/root/.axon_site/_ro/trn_rl_repo"):
    if p not in sys.path:
        sys.path.insert(0, p)

import ml_dtypes
import numpy as np

B, S, DIM, H, HD = 2, 2048, 1024, 16, 64
NCORES = 8
HG = 4  # heads per core
OG = HG * HD  # 256 model dims per core
NB = S // 512  # 4 i-blocks of 512
NJ = S // 128  # 16 j-tiles of 128

LN2 = float(np.log(2.0))
SHIFT = 4.0  # global exp shift, cancels in softmax
A8 = 1.0 / LN2  # fp8 bit-trick slope (on raw psum p = 8*s)
B8 = 56.0 - 8.0 * SHIFT / LN2 - 0.344
A16 = 16.0 / LN2  # bf16 bit-trick slope (128*0.125/ln2)
B16 = 16256.0 - 128.0 * SHIFT / LN2 - 5.5

_CACHE = {}


def _build():
    import concourse.tile as tile
    from concourse import bacc, mybir

    f32 = mybir.dt.float32
    bf16 = mybir.dt.bfloat16
    f8 = mybir.dt.float8e4
    i16 = mybir.dt.int16
    u8 = mybir.dt.uint8
    Exp = mybir.ActivationFunctionType.Exp
    DR = mybir.MatmulPerfMode.DoubleRow
    MULT = mybir.AluOpType.mult
    ADD = mybir.AluOpType.add

    nc = bacc.Bacc("TRN2", target_bir_lowering=False, debug=False, num_devices=NCORES)

    xT = nc.dram_tensor("xT", [DIM, S], bf16, kind="ExternalInput")
    wqT = nc.dram_tensor("wqT", [DIM, OG], bf16, kind="ExternalInput")
    wkT = nc.dram_tensor("wkT", [DIM, OG], bf16, kind="ExternalInput")
    wkT8 = nc.dram_tensor("wkT8", [DIM, OG], f8, kind="ExternalInput")
    wvT = nc.dram_tensor("wvT", [DIM, OG], bf16, kind="ExternalInput")
    wvT8 = nc.dram_tensor("wvT8", [DIM, OG], f8, kind="ExternalInput")
    woT = nc.dram_tensor("woT", [OG, DIM], bf16, kind="ExternalInput")
    woT8 = nc.dram_tensor("woT8", [OG, DIM], f8, kind="ExternalInput")
    cmask8 = nc.dram_tensor("cmask8", [128, 128], f32, kind="ExternalInput")
    m16d = nc.dram_tensor("m16d", [128, 512], f32, kind="ExternalInput")
    y = nc.dram_tensor("y", [S, DIM], bf16, kind="ExternalOutput")

    wqT_r = wqT.ap().rearrange("(t p) o -> t p o", p=128)  # [8,128,256]
    wkT_r = wkT.ap().rearrange("(t p) o -> t p o", p=128)
    wvT_r = wvT.ap().rearrange("(t p) o -> t p o", p=128)
    woT_r = woT.ap().rearrange("(t p) e -> t p e", p=128)  # [2,128,1024]
    y_r = y.ap().rearrange("(t p) e -> t p e", p=128)  # [16,128,1024]
    xT_c = xT.ap().rearrange("(t p) s -> t p s", p=128)
    # fp8 DoubleRow layouts: k-subtile pairs along dim u
    wkT8_c = wkT8.ap().rearrange("(d u p) (m o) -> d m p u o", u=2, p=128, o=128)
    wvT8_c = wvT8.ap().rearrange("(d u p) o -> d p u o", u=2, p=128)
    woT8_c = woT8.ap().rearrange("(u p) (n e) -> n p u e", u=2, p=128, e=512)

    with tile.TileContext(nc) as tc:
        with (
            tc.tile_pool(name="persist", bufs=1) as pp,
            tc.tile_pool(name="work", bufs=4) as wp,
            tc.tile_pool(name="psum", bufs=4, space="PSUM") as ps,
        ):
            # ---- persistent SBUF tiles -------------------------------------
            xtb = [pp.tile([128, S], bf16, tag=f"xt{e}", name=f"xt{e}") for e in range(8)]
            xt2 = [[xtb[e][:, n * 512 : (n + 1) * 512] for n in range(4)] for e in range(8)]
            x8b = [pp.tile([128, 2, S], f8, tag=f"x8{d}", name=f"x8{d}") for d in range(4)]
            x8t = [[x8b[d][:, :, n * 512 : (n + 1) * 512] for n in range(4)] for d in range(4)]
            wqt = [pp.tile([128, OG], bf16, tag=f"wq{i}", name=f"wq{i}") for i in range(8)]
            wk8 = [
                [pp.tile([128, 2, 128], f8, tag=f"wk8{d}_{m}", name=f"wk8{d}_{m}") for m in range(2)]
                for d in range(4)
            ]
            wvt = [pp.tile([128, OG], bf16, tag=f"wv{i}", name=f"wv{i}") for i in range(8)]
            wv8 = [pp.tile([128, 2, OG], f8, tag=f"wv8{d}", name=f"wv8{d}") for d in range(4)]
            wot = [pp.tile([128, DIM], bf16, tag=f"wo{i}", name=f"wo{i}") for i in range(2)]
            wo8 = [pp.tile([128, 2, 512], f8, tag=f"wo8{i}", name=f"wo8{i}") for i in range(2)]
            cm = pp.tile([128, 128], f32, tag="cm")
            m16 = pp.tile([128, 512], f32, tag="m16")
            bias4 = pp.tile([128, 1], f32, tag="bias4")
            kT2 = [pp.tile([128, S], bf16, tag=f"kT{i}", name=f"kT{i}") for i in range(2)]
            qT2 = [pp.tile([128, S], bf16, tag=f"qT{i}", name=f"qT{i}") for i in range(2)]
            qTp = [pp.tile([128, S], bf16, tag=f"qTp{i}", name=f"qTp{i}") for i in range(4)]
            zT2 = [pp.tile([128, 512], bf16, tag=f"zT{i}", name=f"zT{i}") for i in range(2)]
            z8 = pp.tile([128, 12, 2, 128], f8, tag="z8")
            # bf16 V with ones column, per j-tile: [128, head, 65]
            vvb = [pp.tile([128, HG, HD + 1], bf16, tag=f"vvb{i}", name=f"vvb{i}") for i in range(NJ)]
            # fp8 V pairs for DoubleRow: [128, head, 2, 128]; col 64 = ones,
            # cols 65:128 = zeros (dual-fp8 ldweights wants full 128 cols)
            vv8 = [pp.tile([128, HG, 2, 128], f8, tag=f"vv8{i}", name=f"vv8{i}") for i in range(6)]

            qs = [nc.sync, nc.gpsimd, nc.scalar]
            qi = 0

            def q():
                nonlocal qi
                qi += 1
                return qs[qi % 3]

            wkt_full = [pp.tile([128, OG], bf16, tag=f"wkf{i}", name=f"wkf{i}") for i in range(8)]

            # ---- loads (ordered by first use; 3 queues, bulk x DMAs) -------
            for e in range(8):
                q().dma_start(out=xtb[e][:, 0:1024], in_=xT_c[e][:, 0:1024])
                q().dma_start(out=wkt_full[e], in_=wkT_r[e])
                q().dma_start(out=wqt[e], in_=wqT_r[e])
            for e in range(8):
                q().dma_start(out=wvt[e], in_=wvT_r[e])
                q().dma_start(out=xtb[e][:, 1024:2048], in_=xT_c[e][:, 1024:2048])
            for d in range(4):
                for m in range(2):
                    q().dma_start(out=wk8[d][m], in_=wkT8_c[d, m])
                q().dma_start(out=wv8[d], in_=wvT8_c[d])
            for i in range(2):
                q().dma_start(out=wot[i], in_=woT_r[i])
                q().dma_start(out=wo8[i], in_=woT8_c[i])
            nc.gpsimd.dma_start(out=cm, in_=cmask8.ap())
            nc.gpsimd.dma_start(out=m16, in_=m16d.ap())
            nc.vector.memset(bias4, -SHIFT)
            for i in range(6):
                nc.vector.memset(vv8[i], 0.0)
            # zero pads of qTp (the other head's partitions stay 0)
            for h in range(4):
                pad = slice(64, 128) if h % 2 == 0 else slice(0, 64)
                nc.vector.memset(qTp[h][pad, :], 0.0)
            # x8 derived on-chip (DVE idle during the load phase)
            for d in range(4):
                for u in range(2):
                    nc.vector.tensor_copy(x8b[d][:, u, 0:1024], xtb[2 * d + u][:, 0:1024])
            for d in range(4):
                for u in range(2):
                    nc.vector.tensor_copy(x8b[d][:, u, 1024:2048], xtb[2 * d + u][:, 1024:2048])

            def proj_qk(m, n, which):
                acc = ps.tile([128, 512], f32, tag="aux", bufs=2, name="acc")
                wt = wqt if which == "q" else wkt_full
                if which == "q" or n == 0:
                    for e in range(8):
                        nc.tensor.matmul(
                            acc,
                            wt[e][:, m * 128 : (m + 1) * 128],
                            xt2[e][n],
                            start=(e == 0),
                            stop=(e == 7),
                        )
                else:  # fp8 DoubleRow K
                    for d in range(4):
                        nc.tensor.matmul(
                            acc,
                            wk8[d][m],
                            x8t[d][n],
                            start=(d == 0),
                            stop=(d == 3),
                            perf_mode=DR,
                        )
                blk = slice(n * 512, (n + 1) * 512)
                dst = (kT2 if which == "k" else qT2)[m][:, blk]
                if (m + n) % 2 == 0:
                    nc.vector.tensor_copy(dst, acc)
                else:
                    nc.scalar.copy(dst, acc)
                if which == "q":
                    # padded per-head copies on idle DMA queues
                    for hh in (2 * m, 2 * m + 1):
                        rows = slice(0, 64) if hh % 2 == 0 else slice(64, 128)
                        nc.sync.dma_start(out=qTp[hh][rows, blk], in_=qT2[m][rows, blk])

            def proj_v(s):
                acc2 = ps.tile([128, 512], f32, tag="aux", bufs=2, name="acc2")
                av = acc2[:, 0:OG]
                if s < 4:
                    for e in range(8):
                        nc.tensor.matmul(
                            av,
                            xt2[e][s // 4][:, (s % 4) * 128 : (s % 4 + 1) * 128],
                            wvt[e],
                            start=(e == 0),
                            stop=(e == 7),
                        )
                else:
                    for d in range(4):
                        nc.tensor.matmul(
                            av,
                            x8t[d][s // 4][:, :, (s % 4) * 128 : (s % 4 + 1) * 128],
                            wv8[d],
                            start=(d == 0),
                            stop=(d == 3),
                            perf_mode=DR,
                        )
                vsrc = av.rearrange("p (h d) -> p h d", h=HG)
                nc.scalar.copy(vvb[s][:, :, 0:HD], vsrc)
                if s < 12:
                    nc.vector.tensor_copy(vv8[s // 2][:, :, s % 2, 0:HD], vsrc)
                nc.gpsimd.memset(vvb[s][:, :, HD : HD + 1], 1.0)
                if s < 12 and s % 2 == 1:
                    for u in range(2):
                        nc.gpsimd.memset(vv8[s // 2][:, :, u, HD : HD + 1], 1.0)

            # ---- attention -------------------------------------------------
            # exp-engine balance: fraction of plain pairs on Scalar
            sc_acc = [0.0]
            SC_SHARE = 0.90

            def mk_stream(ib, h):
                m = h // 2
                pslc = slice(64 * (h % 2), 64 * (h % 2) + 64)
                iblk = slice(ib * 512, (ib + 1) * 512)
                psz = ps.tile([128, 512], f32, tag="z", bufs=2, name="psz")
                units = []

                for pr in range(2 * ib):  # plain j-tile pairs, fp8 DoubleRow
                    def mk(pr=pr):
                        ex = wp.tile([128, 2, 512], f8, tag="ex8", bufs=12, name="ex8")
                        sc_acc[0] += SC_SHARE
                        on_scalar = sc_acc[0] >= 1.0
                        if on_scalar:
                            sc_acc[0] -= 1.0

                        def se():
                            for u in range(2):
                                psp = ps.tile([128, 512], f32, tag="sc", bufs=4, name="psp")
                                nc.tensor.matmul(
                                    psp,
                                    kT2[m][:, (2 * pr + u) * 128 : (2 * pr + u + 1) * 128],
                                    qTp[h][:, iblk],
                                    start=True,
                                    stop=True,
                                )
                                if on_scalar:
                                    nc.scalar.activation(
                                        ex[:, u, :], psp, Exp, scale=0.125, bias=bias4
                                    )
                                else:
                                    nc.vector.tensor_scalar(
                                        ex.bitcast(u8)[:, u, :], psp, A8, B8, MULT, ADD
                                    )

                        def av():
                            nc.tensor.matmul(
                                psz,
                                vv8[pr][:, h, :, :],
                                ex,
                                start=(pr == 0),
                                stop=False,
                                perf_mode=DR,
                            )

                        return se, av

                    units.append(mk())

                for t in range(4):  # diagonal band tiles, bf16
                    def mk(t=t):
                        jb = 4 * ib + t
                        off = 128 * t
                        w = 512 - off
                        exb = wp.tile([128, 512], i16, tag="exb", bufs=12, name="exb")

                        def se():
                            ps2 = ps.tile([128, 512], f32, tag="sc", bufs=4, name="ps2")
                            nc.tensor.matmul(
                                ps2[:, off:512],
                                kT2[m][:, jb * 128 : (jb + 1) * 128],
                                qTp[h][:, ib * 512 + off : (ib + 1) * 512],
                                start=True,
                                stop=True,
                            )
                            if ib == 0:
                                # precision-critical rows: true exp (bf16 out)
                                nc.vector.tensor_add(
                                    ps2[:, off : off + 128],
                                    ps2[:, off : off + 128],
                                    cm,
                                )
                                nc.scalar.activation(
                                    exb.bitcast(bf16)[:, off:512],
                                    ps2[:, off:512],
                                    Exp,
                                    scale=0.125,
                                    bias=bias4,
                                )
                            else:
                                # mask + exp folded into one DVE op
                                nc.vector.scalar_tensor_tensor(
                                    out=exb[:, off:512],
                                    in0=ps2[:, off:512],
                                    scalar=A16,
                                    in1=m16[:, 0:w],
                                    op0=MULT,
                                    op1=ADD,
                                )

                        def av():
                            nc.tensor.matmul(
                                psz[0:65, off:512],
                                vvb[jb][:, h, :],
                                exb.bitcast(bf16)[:, off:512],
                                start=(ib == 0 and t == 0),
                                stop=(t == 3),
                            )

                        return se, av

                    units.append(mk())

                def norm():
                    dn = wp.tile([1, 512], f32, tag="dn", bufs=4, name="dn")
                    nc.scalar.copy(dn, psz[64:65, :])
                    rc1 = wp.tile([1, 512], f32, tag="rc1", bufs=4, name="rc1")
                    nc.vector.reciprocal_approx_fast(rc1, dn)
                    rcb = wp.tile([64, 512], f32, tag="rcb", bufs=4, name="rcb")
                    nc.gpsimd.partition_broadcast(rcb, rc1)
                    if ib == 0:
                        nc.vector.tensor_mul(zT2[m][pslc, :], psz[0:64, :], rcb)
                    else:
                        nc.vector.tensor_mul(
                            z8[pslc, 4 * ib - 4 : 4 * ib, m, :],
                            psz[0:64, :].rearrange("p (s c) -> p s c", c=128),
                            rcb.rearrange("p (s c) -> p s c", c=128),
                        )

                return units, norm

            def mk_outproj_s(s):
                def go():
                    ysb = wp.tile([128, DIM], bf16, tag="ysb", bufs=3, name="ysb")
                    for n2 in range(2):
                        psy = ps.tile([128, 512], f32, tag="aux", bufs=2, name="psy")
                        if s < 4:
                            for kk in range(2):
                                nc.tensor.matmul(
                                    psy,
                                    zT2[kk][:, s * 128 : (s + 1) * 128],
                                    wot[kk][:, n2 * 512 : (n2 + 1) * 512],
                                    start=(kk == 0),
                                    stop=(kk == 1),
                                )
                        else:
                            nc.tensor.matmul(
                                psy,
                                z8[:, s - 4, :, :],
                                wo8[n2],
                                start=True,
                                stop=True,
                                perf_mode=DR,
                            )
                        half = ysb[:, n2 * 512 : (n2 + 1) * 512]
                        if (s + n2) % 2 == 0:
                            nc.scalar.copy(half, psy)
                        else:
                            nc.vector.tensor_copy(half, psy)
                    nc.sync.dma_start(out=y_r[s], in_=ysb)

                return go

            # cross-stream software pipeline: a global action FIFO keeps the
            # PE's AV/norm/outproj work LAG score-units behind the score/exp
            # emissions so exp results are ready when the PE needs them.
            LAG = 5
            fifo = []

            def push(action):
                fifo.append(action)
                while len(fifo) > LAG:
                    fifo.pop(0)()

            # minimal prologue: only what stream (0,h0) needs
            proj_qk(0, 0, "k")
            proj_qk(0, 0, "q")
            for s in range(4):
                proj_v(s)
            # remaining proj work interleaved into early attention streams
            proj_actions = [
                [lambda: proj_qk(1, 0, "k"), lambda: proj_qk(1, 0, "q")],
                [lambda: proj_qk(0, 1, "k"), lambda: proj_qk(0, 1, "q")],
                [lambda: proj_qk(1, 1, "k"), lambda: proj_qk(1, 1, "q"),
                 lambda: proj_v(4), lambda: proj_v(5)],
                [lambda: proj_qk(0, 2, "k"), lambda: proj_qk(0, 2, "q"),
                 lambda: proj_v(6), lambda: proj_v(7)],
                [lambda: proj_qk(1, 2, "k"), lambda: proj_qk(1, 2, "q"),
                 lambda: proj_v(8), lambda: proj_v(9)],
                [lambda: proj_qk(0, 3, "k"), lambda: proj_qk(0, 3, "q"),
                 lambda: proj_v(10), lambda: proj_v(11)],
                [lambda: proj_qk(1, 3, "k"), lambda: proj_qk(1, 3, "q"),
                 lambda: proj_v(12), lambda: proj_v(13)],
                [lambda: proj_v(14), lambda: proj_v(15)],
            ]
            ready_outproj = []
            sidx = 0
            for ib in range(NB):
                for h in range(HG):
                    units, norm = mk_stream(ib, h)
                    acts = list(proj_actions[sidx]) if sidx < len(proj_actions) else []
                    # feed pending outproj units into the tail streams too
                    while ready_outproj and len(acts) < max(2, len(units) // 3):
                        acts.append(ready_outproj.pop(0))
                    na = len(units)
                    step = max(1, na // (len(acts) + 1)) if acts else na + 1
                    for ui, (se, av) in enumerate(units):
                        se()
                        push(av)
                        if acts and ui % step == step - 1:
                            push(acts.pop(0))
                    for a in acts:
                        push(a)
                    push(norm)
                    sidx += 1
                    if h == 3 and ib >= 1:
                        for s in range(4 * (ib - 1), 4 * ib):
                            ready_outproj.append(mk_outproj_s(s))
            while fifo:
                fifo.pop(0)()
            for a in ready_outproj:
                a()
            for s in range(12, 16):
                mk_outproj_s(s)()

    nc.compile()
    return nc


def _get_nc():
    if "nc" not in _CACHE:
        _CACHE["nc"] = _build()
    return _CACHE["nc"]


def _in_maps(x, mask, wq, wk, wv, wo):
    bf = ml_dtypes.bfloat16
    e4 = ml_dtypes.float8_e4m3
    cm8 = np.ascontiguousarray(8.0 * np.asarray(mask)[0, 0, :128, :128].T, np.float32)
    m16 = np.full((128, 512), B16, np.float32)
    tri = np.asarray(mask)[0, 0, :128, :128].T  # [j, i], -1e9 where j > i
    m16[:, :128] += tri.astype(np.float32)
    maps = []
    for c in range(NCORES):
        b, g = divmod(c, HG)
        sl = slice(OG * g, OG * (g + 1))
        xTb = np.ascontiguousarray(np.asarray(x)[b].T)
        wkTg = np.ascontiguousarray(np.asarray(wk)[sl, :].T)
        wvTg = np.ascontiguousarray(np.asarray(wv)[sl, :].T)
        woTg = np.ascontiguousarray(np.asarray(wo)[:, sl].T)
        maps.append(
            {
                "xT": xTb.astype(bf),
                "wqT": np.ascontiguousarray(np.asarray(wq)[sl, :].T).astype(bf),
                "wkT": wkTg.astype(bf),
                "wkT8": wkTg.astype(e4),
                "wvT": wvTg.astype(bf),
                "wvT8": wvTg.astype(e4),
                "woT": woTg.astype(bf),
                "woT8": woTg.astype(e4),
                "cmask8": cm8,
                "m16d": m16,
            }
        )
    return maps


def _combine(results):
    y = np.zeros((B, S, DIM), np.float32)
    for c in range(NCORES):
        y[c // HG] += results[c]["y"].astype(np.float32)
    return y


def kernel(x, mask, wq, wk, wv, wo, **run_kwargs):
    from concourse.bass_utils import run_bass_kernel_spmd

    nc = _get_nc()
    res = run_bass_kernel_spmd(
        nc, _in_maps(x, mask, wq, wk, wv, wo), core_ids=list(range(NCORES)),
        **run_kwargs,
    )
    out = _combine(res.results)
    if run_kwargs:
        _CACHE["last_result"] = res
    return out
